# revision 6
# baseline (speedup 1.0000x reference)
"""Trainium2 Bass kernel for nn_MicroExpert (sparse_attention).

Reference model (B=2, T=2048, D=512, H=8, HD=64):
  v_in = conv1d(x, k=3, pad=1); MHA(q=x, k=x, v=v_in) with banded mask
  |i-j| <= 256; h = LN(x + attn); out = LN(h + FFN(h)).

Sharding: data-parallel over (batch, 512-token chunk) -> 8 independent
cores, no collectives.  Each core recomputes the K/V halo (+-256 tokens,
zero-padded at sequence edges; pad keys are neutralized via a
denominator correction `padcnt`).

Layout strategy: projections compute K/Q feature-major and V token-major.
The ctx matmul uses ex as the stationary operand so ctx lands TOKEN-major
[q, vd] with the softmax denominator in column 64 of each head's 65-wide
slot (ones-column trick).  Normalization is then a per-head per-partition
scalar multiply.  out_proj and FFN2 keep token-major outputs by streaming
the weight as the moving operand, so residuals + both LayerNorms run
token-major with only two PE transposes per 128-token tile (ctx -> [vd,q]
for out_proj, hn -> [dc,q] for FFN1).  LN stats via DVE bn_stats/bn_aggr;
rstd = exp(-0.5*ln(var+eps)) on ACT (same act table as softmax exp ->
no table reloads).  The conv is folded into the V projection on the host:
v[t] = sum_d U_d @ x[t+d-1], U_d = Wv @ conv_w[:,:,d].
"""

import os
import sys

import numpy as np

sys.path.insert(0, "/opt/trn_rl_repo")

import concourse.bass as bass
import concourse.mybir as mybir
import concourse.tile as tile
from concourse import bacc
from concourse.bass_utils import run_bass_kernel_spmd

BF16 = mybir.dt.bfloat16
F32 = mybir.dt.float32

B, T, D, H, HD = 2, 2048, 512, 8, 64
S = 512          # tokens per core
KV = 1024        # extended kv tokens per core (S + 2*256)
XE = 1026        # x_ext width (KV + 2 for conv halo)
NQT = 4          # 128-query tiles per core
NKT = 5          # relative 128-key tiles per query tile
F = 1024         # FFN hidden
EPS = 1e-5
N_CORES = 8

_cached = {}


def _build_program():
    nc = bacc.Bacc("TRN2", target_bir_lowering=False, debug=False)

    # ---- DRAM tensors -------------------------------------------------
    def din(name, shape, dt):
        return nc.dram_tensor(name, shape, dt, kind="ExternalInput").ap()

    # all inputs are pre-layouted [128, N] SBUF images (host does the packing)
    xt_d = din("xt", [128, 4 * XE], BF16)      # x extended, feature-major
    wk_d = din("wk", [128, 2048], BF16)        # Wk.T (kc,oc) 128x128 blocks
    wq_d = din("wq", [128, 2048], BF16)        # Wq.T (kc,oc) blocks
    wu_d = din("wu", [128, 6144], BF16)        # conv-folded V weights (tap,dc)
    xq_d = din("xq", [128, 2048], BF16)        # x token-major (residual)
    woT_d = din("woT", [128, 2048], BF16)      # Wo.T row-blocks [vd c][512]
    w1_d = din("w1", [128, 4096], BF16)        # w1.T (dc,fc) 128x128 blocks
    w2T_d = din("w2T", [128, 4096], BF16)      # w2.T row-blocks [fc][512]
    mask_d = din("mask01", [128, 256], BF16)   # [tril | triu] 0/1
    padcnt_d = din("padcnt", [128, 4], F32)    # [q-in-tile, qt]
    ident_d = din("ident", [128, 128], BF16)

    out_d = nc.dram_tensor("out", [128, 2048], BF16, kind="ExternalOutput").ap()

    with tile.TileContext(nc) as tc:
        from contextlib import ExitStack

        with ExitStack() as ctx:
            const = ctx.enter_context(tc.tile_pool(name="const", bufs=1))

            # ---- load constants/weights (priority order!) --------------
            def load_w(dram, cols, dt=BF16, parts=128, step=512):
                t = const.tile([parts, cols], dt, name=f"w_{dram.tensor.name}")
                for c0 in range(0, cols, step):
                    c1 = min(cols, c0 + step)
                    nc.sync.dma_start(t[:, c0:c1], dram[:, c0:c1])
                return t

            xt_sb = const.tile([128, 4 * XE], BF16)
            for c in range(4):
                nc.sync.dma_start(
                    xt_sb[:, XE * c: XE * c + XE], xt_d[:, XE * c: XE * c + XE]
                )
            wk_sb = load_w(wk_d, 2048)
            wq_sb = load_w(wq_d, 2048)
            wu_sb = load_w(wu_d, 6144)
            xq_sb = load_w(xq_d, 2048)
            woT_sb = load_w(woT_d, 2048)
            w1_sb = load_w(w1_d, 4096)
            w2T_sb = load_w(w2T_d, 4096)
            mask_sb = load_w(mask_d, 256)
            ident_sb = load_w(ident_d, 128)
            padcnt_sb = load_w(padcnt_d, 4, F32)

            # persistent activations
            kt_sb = const.tile([128, 4 * KV], BF16)    # [oc-block][kv]
            q_sb = const.tile([128, 4 * S], BF16)      # [oc-block][tok]
            v_sb = const.tile([128, 8 * 520], BF16)    # [kv-tok][(v_h|1) x 8]
            hn_sb = const.tile([128, 4 * 512], BF16)   # [tok][qt-block][D]
            h1t_sb = const.tile([128, 4 * 512], BF16)  # [dc-block][tok]

            eps_sb = const.tile([128, 1], F32)
            nc.gpsimd.memset(v_sb[:], 1.0)
            nc.gpsimd.memset(eps_sb[:], float(EPS))
            h1t_v = h1t_sb[:].rearrange("p (c w) -> p c w", c=4)

            # ---- projections: kT, qT, v --------------------------------
            with tc.tile_pool(name="pp", bufs=2, space="PSUM") as pp_pool:
                for oc in range(4):
                    for half in range(2):
                        pp = pp_pool.tile([128, 512], F32, tag="pp")
                        for kc in range(4):
                            nc.tensor.matmul(
                                pp[:],
                                wk_sb[:, 128 * (4 * kc + oc):128 * (4 * kc + oc) + 128],
                                xt_sb[:, XE * kc + 1 + 512 * half: XE * kc + 1 + 512 * half + 512],
                                start=(kc == 0), stop=(kc == 3),
                            )
                        nc.scalar.copy(
                            kt_sb[:, KV * oc + 512 * half: KV * oc + 512 * half + 512], pp[:]
                        )
                for oc in range(4):
                    pp = pp_pool.tile([128, 512], F32, tag="pp")
                    for kc in range(4):
                        nc.tensor.matmul(
                            pp[:],
                            wq_sb[:, 128 * (4 * kc + oc):128 * (4 * kc + oc) + 128],
                            xt_sb[:, XE * kc + 257: XE * kc + 257 + 512],
                            start=(kc == 0), stop=(kc == 3),
                        )
                    nc.vector.tensor_copy(q_sb[:, 512 * oc: 512 * oc + 512], pp[:])
                for tt in range(8):
                    pp = pp_pool.tile([128, 512], F32, tag="pp")
                    n = 0
                    for tap in range(3):
                        for dc in range(4):
                            nc.tensor.matmul(
                                pp[:],
                                xt_sb[:, XE * dc + 128 * tt + tap: XE * dc + 128 * tt + tap + 128],
                                wu_sb[:, 512 * (4 * tap + dc): 512 * (4 * tap + dc) + 512],
                                start=(n == 0), stop=(n == 11),
                            )
                            n += 1
                    vv = v_sb[:, 520 * tt: 520 * tt + 520].rearrange(
                        "p (h w) -> p h w", h=8
                    )
                    nc.scalar.copy(vv[:, :, 0:64], pp[:].rearrange("p (h w) -> p h w", h=8))

            # ---- attention + LN1, per query tile -----------------------
            with ExitStack() as actx:
                sc_pool = actx.enter_context(tc.tile_pool(name="scps", bufs=2, space="PSUM"))
                cx_pool = actx.enter_context(tc.tile_pool(name="cxps", bufs=1, space="PSUM"))
                tp_pool = actx.enter_context(tc.tile_pool(name="tpps", bufs=1, space="PSUM"))
                at_pool = actx.enter_context(tc.tile_pool(name="atps", bufs=1, space="PSUM"))
                ex_pool = actx.enter_context(tc.tile_pool(name="exsb", bufs=3))
                cn_pool = actx.enter_context(tc.tile_pool(name="cnsb", bufs=2))
                r1_pool = actx.enter_context(tc.tile_pool(name="r1sb", bufs=2))
                sm_pool = actx.enter_context(tc.tile_pool(name="smsb", bufs=4))

                # head h ctx slot: 4 heads per psum bank so no 65-wide slot
                # straddles a 512-col bank boundary
                hcol = lambda h: 512 * (h // 4) + 65 * (h % 4)

                for qt in range(NQT):
                    cxps = cx_pool.tile([128, 1024], F32, tag="cx")
                    for h in range(8):
                        oc, hp = h // 2, 64 * (h % 2)
                        scps = sc_pool.tile([128, 640], F32, tag="sc")
                        for kt in range(NKT):
                            kcol = KV * oc + 128 * (qt + kt)
                            nc.tensor.matmul(
                                scps[:, 128 * kt: 128 * kt + 128],
                                kt_sb[hp:hp + 64, kcol:kcol + 128],
                                q_sb[hp:hp + 64, 512 * oc + 128 * qt: 512 * oc + 128 * qt + 128],
                                start=True, stop=True,
                            )
                        ex = ex_pool.tile([128, 640], BF16, tag="ex")
                        nc.scalar.activation(
                            ex[:], scps[:], mybir.ActivationFunctionType.Exp,
                            scale=float(1.0 / np.sqrt(HD)),
                        )
                        # unmasked ctx tiles start right after the exp
                        for kt in (1, 2, 3):
                            nc.tensor.matmul(
                                cxps[:, hcol(h): hcol(h) + 65],
                                ex[:, 128 * kt: 128 * kt + 128],
                                v_sb[:, 520 * (qt + kt) + 65 * h: 520 * (qt + kt) + 65 * h + 65],
                                start=(kt == 1), stop=False,
                                skip_group_check=True,
                            )
                        # band mask on relative tiles 0 and 4 (0/1 mult), Pool
                        ex_edge = ex[:].rearrange("p (a b) -> p a b", a=5)[:, ::4, :]
                        nc.gpsimd.tensor_mul(
                            ex_edge, ex_edge,
                            mask_sb[:].rearrange("p (n w) -> p n w", n=2),
                        )
                        for kt in (0, 4):
                            nc.tensor.matmul(
                                cxps[:, hcol(h): hcol(h) + 65],
                                ex[:, 128 * kt: 128 * kt + 128],
                                v_sb[:, 520 * (qt + kt) + 65 * h: 520 * (qt + kt) + 65 * h + 65],
                                start=False, stop=(kt == 4),
                                skip_group_check=True,
                            )
                    # denominators: col 64 of each head slot, minus pad count
                    den = sm_pool.tile([128, 8], F32, tag="den")
                    denv = den[:].rearrange("p (h w) -> p h w", w=1)
                    for g in range(2):
                        cxg = cxps[:, 512 * g: 512 * g + 260].rearrange(
                            "p (h w) -> p h w", h=4
                        )
                        nc.vector.tensor_scalar_sub(
                            denv[:, 4 * g: 4 * g + 4, :],
                            cxg[:, :, 64:65],
                            padcnt_sb[:, qt:qt + 1],
                        )
                    rden = sm_pool.tile([128, 8], F32, tag="rden")
                    nc.vector.reciprocal(rden[:], den[:])
                    # normalize ctx -> token-major [q, vd] bf16
                    ctxn = cn_pool.tile([128, 512], BF16, tag="ctxn")
                    for h in range(8):
                        nc.vector.tensor_scalar_mul(
                            ctxn[:, 64 * h: 64 * h + 64],
                            cxps[:, hcol(h): hcol(h) + 64],
                            rden[:, h:h + 1],
                        )
                    # transpose -> [vd, q] for out_proj
                    tpps = tp_pool.tile([128, 512], BF16, tag="tp")
                    for c in range(4):
                        nc.tensor.transpose(
                            tpps[:, 128 * c: 128 * c + 128],
                            ctxn[:, 128 * c: 128 * c + 128],
                            ident_sb[:],
                        )
                    ctxT = cn_pool.tile([128, 512], BF16, tag="ctxT")
                    nc.vector.tensor_copy(ctxT[:], tpps[:])
                    # out_proj (token-major out) + residual -> r1 [q, D]
                    atps = at_pool.tile([128, 512], F32, tag="at")
                    for c in range(4):
                        nc.tensor.matmul(
                            atps[:],
                            ctxT[:, 128 * c: 128 * c + 128],
                            woT_sb[:, 512 * c: 512 * c + 512],
                            start=(c == 0), stop=(c == 3),
                        )
                    r1 = r1_pool.tile([128, 512], BF16, tag="r1")
                    nc.vector.tensor_add(
                        r1[:], atps[:], xq_sb[:, 512 * qt: 512 * qt + 512]
                    )
                    # LN1: bn_stats/bn_aggr; rstd = exp(-0.5*ln(var+eps))
                    bn6 = sm_pool.tile([128, 6], F32, tag="bn6")
                    nc.vector.bn_stats(bn6[:], r1[:])
                    mv = sm_pool.tile([128, 2], F32, tag="mv")
                    nc.vector.bn_aggr(mv[:], bn6[:])
                    rstd = sm_pool.tile([128, 2], F32, tag="rstd")
                    nc.scalar.activation(
                        rstd[:, 0:1], mv[:, 1:2],
                        mybir.ActivationFunctionType.Ln, bias=eps_sb[:, 0:1],
                    )
                    nc.scalar.activation(
                        rstd[:, 1:2], rstd[:, 0:1],
                        mybir.ActivationFunctionType.Exp, scale=-0.5,
                    )
                    nc.vector.tensor_scalar(
                        hn_sb[:, 512 * qt: 512 * qt + 512], r1[:],
                        mv[:, 0:1], rstd[:, 1:2],
                        op0=mybir.AluOpType.subtract, op1=mybir.AluOpType.mult,
                    )
                    # transpose hn -> feature-major for FFN1
                    tpps = tp_pool.tile([128, 512], BF16, tag="tp")
                    for c in range(4):
                        nc.tensor.transpose(
                            tpps[:, 128 * c: 128 * c + 128],
                            hn_sb[:, 512 * qt + 128 * c: 512 * qt + 128 * c + 128],
                            ident_sb[:],
                        )
                    nc.vector.tensor_copy(
                        h1t_v[:, :, 128 * qt: 128 * qt + 128], tpps[:]
                    )

            # ---- FFN + LN2 + store, per query tile ---------------------
            with ExitStack() as fctx:
                m1_pool = fctx.enter_context(tc.tile_pool(name="m1ps", bufs=2, space="PSUM"))
                f2_pool = fctx.enter_context(tc.tile_pool(name="f2ps", bufs=2, space="PSUM"))
                ms_pool = fctx.enter_context(tc.tile_pool(name="m1sb", bufs=2))
                r2_pool = fctx.enter_context(tc.tile_pool(name="r2sb", bufs=2))
                ot_pool = fctx.enter_context(tc.tile_pool(name="otsb", bufs=2))
                s2_pool = fctx.enter_context(tc.tile_pool(name="s2sb", bufs=4))

                for qt in range(NQT):
                    m1sb = ms_pool.tile([128, 1024], BF16, tag="m1")
                    for g in range(2):
                        m1ps = m1_pool.tile([128, 512], F32, tag="m1p")
                        for i in range(4):
                            fc = 4 * g + i
                            for dc in range(4):
                                nc.tensor.matmul(
                                    m1ps[:, 128 * i: 128 * i + 128],
                                    w1_sb[:, 128 * (8 * dc + fc): 128 * (8 * dc + fc) + 128],
                                    h1t_v[:, dc, 128 * qt: 128 * qt + 128],
                                    start=(dc == 0), stop=(dc == 3),
                                )
                        nc.vector.tensor_scalar_max(
                            m1sb[:, 512 * g: 512 * g + 512], m1ps[:], 0.0
                        )
                    f2ps = f2_pool.tile([128, 512], F32, tag="f2")
                    for fcb in range(8):
                        nc.tensor.matmul(
                            f2ps[:],
                            m1sb[:, 128 * fcb: 128 * fcb + 128],
                            w2T_sb[:, 512 * fcb: 512 * fcb + 512],
                            start=(fcb == 0), stop=(fcb == 7),
                        )
                    r2 = r2_pool.tile([128, 512], BF16, tag="r2")
                    nc.vector.tensor_add(
                        r2[:], f2ps[:], hn_sb[:, 512 * qt: 512 * qt + 512]
                    )
                    bn6 = s2_pool.tile([128, 6], F32, tag="bn6b")
                    nc.vector.bn_stats(bn6[:], r2[:])
                    mv = s2_pool.tile([128, 2], F32, tag="mvb")
                    nc.vector.bn_aggr(mv[:], bn6[:])
                    rstd = s2_pool.tile([128, 2], F32, tag="rstdb")
                    nc.scalar.activation(
                        rstd[:, 0:1], mv[:, 1:2],
                        mybir.ActivationFunctionType.Ln, bias=eps_sb[:, 0:1],
                    )
                    nc.scalar.activation(
                        rstd[:, 1:2], rstd[:, 0:1],
                        mybir.ActivationFunctionType.Exp, scale=-0.5,
                    )
                    outt = ot_pool.tile([128, 512], BF16, tag="out")
                    nc.vector.tensor_scalar(
                        outt[:], r2[:],
                        mv[:, 0:1], rstd[:, 1:2],
                        op0=mybir.AluOpType.subtract, op1=mybir.AluOpType.mult,
                    )
                    nc.sync.dma_start(out_d[:, 512 * qt: 512 * qt + 512], outt[:])

    nc.compile()
    return nc


def _prep_host(inputs):
    x = np.asarray(inputs["x"], np.float32)
    conv_w = np.asarray(inputs["conv_w"], np.float32)
    conv_b = np.asarray(inputs["conv_b"], np.float32)
    in_w = np.asarray(inputs["in_proj_w"], np.float32)
    in_b = np.asarray(inputs["in_proj_b"], np.float32)
    out_w = np.asarray(inputs["out_proj_w"], np.float32)
    out_b = np.asarray(inputs["out_proj_b"], np.float32)
    w1 = np.asarray(inputs["w1"], np.float32)
    b1 = np.asarray(inputs["b1"], np.float32)
    w2 = np.asarray(inputs["w2"], np.float32)
    b2 = np.asarray(inputs["b2"], np.float32)
    g1 = np.asarray(inputs["ln1_g"], np.float32)
    bb1 = np.asarray(inputs["ln1_b"], np.float32)
    g2 = np.asarray(inputs["ln2_g"], np.float32)
    bb2 = np.asarray(inputs["ln2_b"], np.float32)

    for nm, v in (("conv_b", conv_b), ("in_proj_b", in_b), ("out_proj_b", out_b),
                  ("b1", b1), ("b2", b2)):
        if np.any(v != 0):
            raise NotImplementedError(f"nonzero {nm} unsupported")
    if np.any(g1 != 1) or np.any(bb1 != 0) or np.any(g2 != 1) or np.any(bb2 != 0):
        raise NotImplementedError("nontrivial layernorm affine unsupported")

    Wq, Wk, Wv = in_w[:D], in_w[D:2 * D], in_w[2 * D:]
    U = [(Wv @ conv_w[:, :, d]) for d in range(3)]  # v[t] = sum U_d @ x[t+d-1]

    def img(stack):  # [n, 128, w] slices -> [128, n*w] SBUF image
        a = np.asarray(stack, np.float32)
        return np.ascontiguousarray(a.transpose(1, 0, 2).reshape(128, -1))

    def slc16(W):  # W used as out = W @ x  -> lhsT slices of W.T
        WT = np.ascontiguousarray(W.T)
        return img([
            WT[128 * kc:128 * kc + 128, 128 * oc:128 * oc + 128]
            for kc in range(4) for oc in range(4)
        ])

    wk_a = slc16(Wk)
    wq_a = slc16(Wq)
    wu_a = img([
        np.ascontiguousarray(U[tap].T)[128 * dc:128 * dc + 128, :]
        for tap in range(3) for dc in range(4)
    ])
    woT_a = img([out_w.T[128 * c:128 * c + 128, :] for c in range(4)])
    w1_a = img([
        np.ascontiguousarray(w1.T)[128 * dc:128 * dc + 128, 128 * fc:128 * fc + 128]
        for dc in range(4) for fc in range(8)
    ])
    w2T_a = img([w2.T[128 * fcb:128 * fcb + 128, :] for fcb in range(8)])

    r = np.arange(128)
    m_lo = (r[:, None] >= r[None, :]).astype(np.float32)   # block 0: keep k>=q
    mask01 = np.concatenate([m_lo, m_lo.T], axis=1)

    ident = np.eye(128, dtype=np.float32)

    def bf(a):
        import ml_dtypes
        return np.asarray(a, dtype=ml_dtypes.bfloat16)

    common = {
        "wk": bf(wk_a), "wq": bf(wq_a), "wu": bf(wu_a), "woT": bf(woT_a),
        "w1": bf(w1_a), "w2T": bf(w2T_a), "mask01": bf(mask01),
        "ident": bf(ident),
    }

    in_maps = []
    for c in range(N_CORES):
        b, j = divmod(c, 4)
        s = 512 * j
        xe = np.zeros((XE, D), np.float32)
        lo, hi = max(0, s - 257), min(T, s + 769)
        xe[lo - (s - 257): hi - (s - 257)] = x[b, lo:hi]
        xt = xe.T.reshape(4, 128, XE).transpose(1, 0, 2).reshape(128, 4 * XE)
        xt = np.ascontiguousarray(xt)

        xq = np.ascontiguousarray(
            x[b, s:s + 512].reshape(4, 128, 512).transpose(1, 0, 2).reshape(128, 2048)
        )

        # padcnt[qt, r]: in-band-kept pad keys
        key = (s - 256 + 128 * np.arange(4)[:, None, None]
               + np.arange(640)[None, None, :])          # [qt,1,640]
        pad = (key < 0) | (key >= T)
        cc, rr = np.arange(640)[None, None, :], r[None, :, None]
        kept = ((cc >= 128) & (cc < 512)) | ((cc < 128) & (cc >= rr)) \
            | ((cc >= 512) & (cc - 512 <= rr))
        pc = (pad & kept).sum(axis=2).astype(np.float32)  # [4, 128]
        padcnt = np.ascontiguousarray(pc.T)               # [128, 4]

        m = dict(common)
        m["xt"] = bf(xt)
        m["xq"] = bf(xq)
        m["padcnt"] = padcnt
        in_maps.append(m)
    return in_maps


def kernel(**inputs) -> np.ndarray:
    if "nc" not in _cached:
        _cached["nc"] = _build_program()
    nc = _cached["nc"]
    in_maps = _prep_host(inputs)
    res = run_bass_kernel_spmd(nc, in_maps, core_ids=list(range(N_CORES)))
    out = np.empty((B, T, D), np.float32)
    for c in range(N_CORES):
        b, j = divmod(c, 4)
        o = np.asarray(res.results[c]["out"], np.float32)
        o = o.reshape(128, 4, 512).transpose(1, 0, 2)
        out[b, 512 * j: 512 * j + 512] = o.reshape(512, 512)
    return out


# revision 10
# speedup vs baseline: 1.2219x; 1.2219x over previous
"""Trainium2 Bass kernel for nn_MicroExpert (sparse_attention).

Reference model (B=2, T=2048, D=512, H=8, HD=64):
  v_in = conv1d(x, k=3, pad=1); MHA(q=x, k=x, v=v_in) with banded mask
  |i-j| <= 256; h = LN(x + attn); out = LN(h + FFN(h)).

Sharding: data-parallel over (batch, 512-token chunk) -> 8 independent
cores, no collectives.  Each core recomputes the K/V halo (+-256 tokens,
zero-padded at sequence edges; pad keys are neutralized via a
denominator correction `padcnt`).

Key implementation points:
- K/Q/V projections run in fp8e4 DoubleRow (2 contraction tiles per
  instruction); per-tensor scales are folded into the psum->sbuf
  copies, so attention math downstream is plain bf16.
- The band mask is ADDED into the score psum as a -240 constant via two
  extra PE matmuls (lhsT=maskT, rhs=identity) closing each edge tile's
  accumulation group -- no vector-engine mask op, exp(score-30)~=0.
- The ctx matmul uses ex as the stationary operand so ctx lands
  TOKEN-major [q, vd] with the softmax denominator in column 64 of each
  head's 65-wide slot (ones-column trick); slots are packed 4-per-psum
  bank so no matmul output straddles a bank.  Normalization is one DVE
  multiply with a broadcast reciprocal-denominator operand.
- out_proj and FFN2 keep token-major outputs by streaming the weight as
  the moving operand, so residuals and both LayerNorms run token-major
  with only two PE transposes per 128-token tile.
- LN stats via DVE bn_stats/bn_aggr; rstd = exp(-0.5*ln(var+eps)) on
  ACT.  An explicit LoadActFuncSet pins the one table that holds Exp,
  Ln and Copy together so no table reloads occur mid-kernel.
- FFN(qt-1) is software-pipelined behind attention(qt); LN2 + store
  happen per query tile so the store overlaps compute.
- The conv is folded into the V projection on the host:
  v[t] = sum_d U_d @ x[t+d-1], U_d = Wv @ conv_w[:,:,d].
"""

import os
import sys

import numpy as np

sys.path.insert(0, "/opt/trn_rl_repo")

import concourse.bass as bass
import concourse.mybir as mybir
import concourse.tile as tile
from concourse import bacc
from concourse.bass_utils import run_bass_kernel_spmd

BF16 = mybir.dt.bfloat16
F32 = mybir.dt.float32
FP8 = mybir.dt.float8e4

B, T, D, H, HD = 2, 2048, 512, 8, 64
S = 512          # tokens per core
KV = 1024        # extended kv tokens per core (S + 2*256)
XE = 1026        # x_ext width (KV + 2 for conv halo)
XE8 = 1152       # fp8 image row stride: DoubleRow planes need stride % 128 == 0
NQT = 4          # 128-query tiles per core
NKT = 5          # relative 128-key tiles per query tile
F = 1024         # FFN hidden
EPS = 1e-5
N_CORES = 8

# fp8 quantization scales (powers of two; values stay well under 240)
SX = 32.0        # x
SWK = 1024.0     # Wk/Wq
SWU = 4096.0     # conv-folded V weight
ACT_TABLE_EXP_LN = 6   # act_info.json index of natural_log_exp_and_others

_cached = {}


def _build_program():
    nc = bacc.Bacc("TRN2", target_bir_lowering=False, debug=False)

    def din(name, shape, dt):
        return nc.dram_tensor(name, shape, dt, kind="ExternalInput").ap()

    # all inputs are pre-layouted [128, N] SBUF images (host does the packing)
    xt_d = din("xt8", [128, 4 * XE8], FP8)     # x extended, feature-major, *SX
    wk_d = din("wk8", [128, 2048], FP8)        # Wk.T (kc,oc) blocks, *SWK
    wq_d = din("wq8", [128, 2048], FP8)        # Wq.T (kc,oc) blocks, *SWK
    wu_d = din("wu8", [128, 6144], FP8)        # conv-folded V w (tap,dc), *SWU
    xq_d = din("xq", [128, 2048], BF16)        # x token-major (residual)
    woT_d = din("woT", [128, 2048], BF16)      # Wo.T row-blocks [vd c][512]
    w1_d = din("w1", [128, 4096], BF16)        # w1.T (dc,fc) 128x128 blocks
    w2T_d = din("w2T", [128, 4096], BF16)      # w2.T row-blocks [fc][512]
    negm_d = din("negm", [128, 256], BF16)     # -240 * (1-keep).T, 2 blocks
    padcnt_d = din("padcnt", [128, 4], F32)    # [q-in-tile, qt]
    ident_d = din("ident", [128, 128], BF16)

    out_d = nc.dram_tensor("out", [128, 2048], BF16, kind="ExternalOutput").ap()

    with tile.TileContext(nc) as tc:
        from contextlib import ExitStack

        with ExitStack() as ctx:
            const = ctx.enter_context(tc.tile_pool(name="const", bufs=1))

            # pin the act table that covers Exp+Ln+Copy for the whole kernel
            nc.scalar.add_instruction(mybir.InstLoadActFuncSet(
                name=nc.get_next_instruction_name(), ins=[], outs=[],
                act_func_set_id=ACT_TABLE_EXP_LN,
            ))

            # ---- loads: one DMA per tensor, split over both HWDGE engines,
            # in consumption order so compute starts as soon as possible
            def load_w(dram, cols, dt=BF16, eng=None):
                t = const.tile([128, cols], dt, name=f"w_{dram.tensor.name}")
                (eng or nc.sync).dma_start(t[:, :], dram[:, :])
                return t

            xt_sb = load_w(xt_d, 4 * XE8, FP8, nc.scalar)
            wk_sb = load_w(wk_d, 2048, FP8, nc.sync)
            wq_sb = load_w(wq_d, 2048, FP8, nc.sync)
            wu_sb = load_w(wu_d, 6144, FP8, nc.scalar)
            ident_sb = load_w(ident_d, 128, BF16, nc.sync)
            negm_sb = load_w(negm_d, 256, BF16, nc.sync)
            padcnt_sb = load_w(padcnt_d, 4, F32, nc.sync)
            xq_sb = load_w(xq_d, 2048, BF16, nc.scalar)
            woT_sb = load_w(woT_d, 2048, BF16, nc.sync)
            w1_sb = load_w(w1_d, 4096, BF16, nc.scalar)
            w2T_sb = load_w(w2T_d, 4096, BF16, nc.scalar)

            # persistent activations
            kt_sb = const.tile([128, 4 * KV], BF16)    # [oc-block][kv]
            q_sb = const.tile([128, 4 * S], BF16)      # [oc-block][tok]
            v_sb = const.tile([128, 8 * 520], BF16)    # [kv-tok][(v_h|1) x 8]
            hn_sb = const.tile([128, 4 * 512], BF16)   # [tok][qt-block][D]
            h1t_sb = const.tile([128, 4 * 512], BF16)  # [dc-block][tok]

            eps_sb = const.tile([128, 1], F32)
            nc.gpsimd.memset(v_sb[:], 1.0)
            nc.gpsimd.memset(eps_sb[:], float(EPS))
            h1t_v = h1t_sb[:].rearrange("p (c w) -> p c w", c=4)

            vx8 = xt_sb[:].rearrange("p (c w) -> p c w", c=4)      # [128,4,XE8]
            vwk = wk_sb[:].rearrange("p (k o w) -> p k o w", k=4, o=4)
            vwq = wq_sb[:].rearrange("p (k o w) -> p k o w", k=4, o=4)
            vwu = wu_sb[:].rearrange("p (t w) -> p t w", t=12)     # [128,12,512]
            DR = mybir.MatmulPerfMode.DoubleRow

            # ---- projections (fp8 DoubleRow): kT, qT, v ----------------
            with tc.tile_pool(name="pp", bufs=2, space="PSUM") as pp_pool:
                for oc in range(4):
                    for half in range(2):
                        pp = pp_pool.tile([128, 512], F32, tag="pp")
                        for g in range(2):
                            nc.tensor.matmul(
                                pp[:],
                                vwk[:, 2 * g: 2 * g + 2, oc, :],
                                vx8[:, 2 * g: 2 * g + 2,
                                    1 + 512 * half: 513 + 512 * half],
                                start=(g == 0), stop=(g == 1), perf_mode=DR,
                            )
                        nc.scalar.mul(
                            kt_sb[:, KV * oc + 512 * half: KV * oc + 512 * half + 512],
                            pp[:], 1.0 / (SX * SWK),
                        )
                for oc in range(4):
                    pp = pp_pool.tile([128, 512], F32, tag="pp")
                    for g in range(2):
                        nc.tensor.matmul(
                            pp[:],
                            vwq[:, 2 * g: 2 * g + 2, oc, :],
                            vx8[:, 2 * g: 2 * g + 2, 257:769],
                            start=(g == 0), stop=(g == 1), perf_mode=DR,
                        )
                    nc.vector.tensor_scalar_mul(
                        q_sb[:, 512 * oc: 512 * oc + 512], pp[:], 1.0 / (SX * SWK)
                    )
                for tt in range(8):
                    pp = pp_pool.tile([128, 512], F32, tag="pp")
                    n = 0
                    for tap in range(3):
                        for i in range(2):
                            nc.tensor.matmul(
                                pp[:],
                                vx8[:, 2 * i: 2 * i + 2,
                                    128 * tt + tap: 128 * tt + tap + 128],
                                vwu[:, 4 * tap + 2 * i: 4 * tap + 2 * i + 2, :],
                                start=(n == 0), stop=(n == 5), perf_mode=DR,
                            )
                            n += 1
                    vv = v_sb[:, 520 * tt: 520 * tt + 520].rearrange(
                        "p (h w) -> p h w", h=8
                    )
                    nc.scalar.mul(
                        vv[:, :, 0:64],
                        pp[:].rearrange("p (h w) -> p h w", h=8),
                        1.0 / (SX * SWU),
                    )

            # ---- attention + LN1 (+ software-pipelined FFN) ------------
            with ExitStack() as actx:
                sc_pool = actx.enter_context(tc.tile_pool(name="scps", bufs=2, space="PSUM"))
                cx_pool = actx.enter_context(tc.tile_pool(name="cxps", bufs=1, space="PSUM"))
                tp_pool = actx.enter_context(tc.tile_pool(name="tpps", bufs=1, space="PSUM"))
                ps_pool = actx.enter_context(tc.tile_pool(name="ps1", bufs=2, space="PSUM"))
                ex_pool = actx.enter_context(tc.tile_pool(name="exsb", bufs=3))
                cn_pool = actx.enter_context(tc.tile_pool(name="cnsb", bufs=2))
                r1_pool = actx.enter_context(tc.tile_pool(name="r1sb", bufs=2))
                sm_pool = actx.enter_context(tc.tile_pool(name="smsb", bufs=4))
                ms_pool = actx.enter_context(tc.tile_pool(name="m1sb", bufs=2))
                r2_pool = actx.enter_context(tc.tile_pool(name="r2sb", bufs=2))
                ot_pool = actx.enter_context(tc.tile_pool(name="otsb", bufs=2))

                def attn_qt(qt):
                    ctxn = cn_pool.tile([128, 512], BF16, tag="ctxn")
                    for g in range(2):
                        cxps = cx_pool.tile([128, 260], F32, tag="cx")
                        for h4 in range(4):
                            h = 4 * g + h4
                            oc, hp = h // 2, 64 * (h % 2)
                            scps = sc_pool.tile([128, 640], F32, tag="sc")
                            for kt in range(NKT):
                                edge = kt in (0, 4)
                                kcol = KV * oc + 128 * (qt + kt)
                                nc.tensor.matmul(
                                    scps[:, 128 * kt: 128 * kt + 128],
                                    kt_sb[hp:hp + 64, kcol:kcol + 128],
                                    q_sb[hp:hp + 64, 512 * oc + 128 * qt: 512 * oc + 128 * qt + 128],
                                    start=True, stop=not edge,
                                )
                                if edge:  # add -240 band mask into the psum group
                                    nc.tensor.matmul(
                                        scps[:, 128 * kt: 128 * kt + 128],
                                        negm_sb[:, 128 * (kt // 4): 128 * (kt // 4) + 128],
                                        ident_sb[:],
                                        start=False, stop=True,
                                    )
                            ex = ex_pool.tile([128, 640], BF16, tag="ex")
                            nc.scalar.activation(
                                ex[:], scps[:], mybir.ActivationFunctionType.Exp,
                                scale=float(1.0 / np.sqrt(HD)),
                            )
                            for kt in range(NKT):
                                nc.tensor.matmul(
                                    cxps[:, 65 * h4: 65 * h4 + 65],
                                    ex[:, 128 * kt: 128 * kt + 128],
                                    v_sb[:, 520 * (qt + kt) + 65 * h: 520 * (qt + kt) + 65 * h + 65],
                                    start=(kt == 0), stop=(kt == 4),
                                    skip_group_check=True,
                                )
                        # denominators (col 64 per slot) minus pad count
                        cxv = cxps[:].rearrange("p (h w) -> p h w", h=4)
                        den = sm_pool.tile([128, 4], F32, tag="den")
                        nc.vector.tensor_scalar_sub(
                            den[:].rearrange("p (h w) -> p h w", w=1),
                            cxv[:, :, 64:65], padcnt_sb[:, qt:qt + 1],
                        )
                        rden = sm_pool.tile([128, 4], F32, tag="rden")
                        nc.vector.reciprocal(rden[:], den[:])
                        # normalize this half -> token-major [q, vd] bf16
                        rdv = rden[:].rearrange(
                            "p (h w) -> p h w", w=1
                        ).to_broadcast((128, 4, 64))
                        nc.vector.tensor_mul(
                            ctxn[:, 256 * g: 256 * g + 256].rearrange(
                                "p (h w) -> p h w", h=4),
                            cxv[:, :, 0:64], rdv,
                        )
                    # transpose -> [vd, q] for out_proj
                    tpps = tp_pool.tile([128, 1024], BF16, tag="tp")
                    for c in range(4):
                        nc.tensor.transpose(
                            tpps[:, 128 * c: 128 * c + 128],
                            ctxn[:, 128 * c: 128 * c + 128],
                            ident_sb[:],
                        )
                    ctxT = cn_pool.tile([128, 512], BF16, tag="ctxT")
                    nc.scalar.copy(ctxT[:], tpps[:, 0:512])
                    # out_proj (token-major out) + residual -> r1 [q, D]
                    atps = ps_pool.tile([128, 512], F32, tag="ps")
                    for c in range(4):
                        nc.tensor.matmul(
                            atps[:],
                            ctxT[:, 128 * c: 128 * c + 128],
                            woT_sb[:, 512 * c: 512 * c + 512],
                            start=(c == 0), stop=(c == 3),
                        )
                    r1 = r1_pool.tile([128, 512], BF16, tag="r1")
                    nc.vector.tensor_add(
                        r1[:], atps[:], xq_sb[:, 512 * qt: 512 * qt + 512]
                    )
                    # LN1: bn_stats/bn_aggr; rstd = exp(-0.5*ln(var+eps))
                    bn6 = sm_pool.tile([128, 6], F32, tag="bn6")
                    nc.vector.bn_stats(bn6[:], r1[:])
                    mv = sm_pool.tile([128, 2], F32, tag="mv")
                    nc.vector.bn_aggr(mv[:], bn6[:])
                    rstd = sm_pool.tile([128, 2], F32, tag="rstd")
                    nc.scalar.activation(
                        rstd[:, 0:1], mv[:, 1:2],
                        mybir.ActivationFunctionType.Ln, bias=eps_sb[:, 0:1],
                    )
                    nc.scalar.activation(
                        rstd[:, 1:2], rstd[:, 0:1],
                        mybir.ActivationFunctionType.Exp, scale=-0.5,
                    )
                    nc.vector.tensor_scalar(
                        hn_sb[:, 512 * qt: 512 * qt + 512], r1[:],
                        mv[:, 0:1], rstd[:, 1:2],
                        op0=mybir.AluOpType.subtract, op1=mybir.AluOpType.mult,
                    )
                    # transpose hn -> feature-major for FFN1 (same tp tile,
                    # upper half)
                    for c in range(4):
                        nc.tensor.transpose(
                            tpps[:, 512 + 128 * c: 512 + 128 * c + 128],
                            hn_sb[:, 512 * qt + 128 * c: 512 * qt + 128 * c + 128],
                            ident_sb[:],
                        )
                    nc.vector.tensor_copy(
                        h1t_v[:, :, 128 * qt: 128 * qt + 128], tpps[:, 512:1024]
                    )

                def ffn_qt(qt):
                    m1sb = ms_pool.tile([128, 1024], BF16, tag="m1")
                    for g in range(2):
                        m1ps = ps_pool.tile([128, 512], F32, tag="ps")
                        for i in range(4):
                            fc = 4 * g + i
                            for dc in range(4):
                                nc.tensor.matmul(
                                    m1ps[:, 128 * i: 128 * i + 128],
                                    w1_sb[:, 128 * (8 * dc + fc): 128 * (8 * dc + fc) + 128],
                                    h1t_v[:, dc, 128 * qt: 128 * qt + 128],
                                    start=(dc == 0), stop=(dc == 3),
                                )
                        nc.vector.tensor_scalar_max(
                            m1sb[:, 512 * g: 512 * g + 512], m1ps[:], 0.0
                        )
                    f2ps = ps_pool.tile([128, 512], F32, tag="ps")
                    for fcb in range(8):
                        nc.tensor.matmul(
                            f2ps[:],
                            m1sb[:, 128 * fcb: 128 * fcb + 128],
                            w2T_sb[:, 512 * fcb: 512 * fcb + 512],
                            start=(fcb == 0), stop=(fcb == 7),
                        )
                    r2 = r2_pool.tile([128, 512], BF16, tag="r2")
                    nc.vector.tensor_add(
                        r2[:], f2ps[:], hn_sb[:, 512 * qt: 512 * qt + 512]
                    )
                    bn6 = sm_pool.tile([128, 6], F32, tag="bn6b")
                    nc.vector.bn_stats(bn6[:], r2[:])
                    mv = sm_pool.tile([128, 2], F32, tag="mvb")
                    nc.vector.bn_aggr(mv[:], bn6[:])
                    rstd = sm_pool.tile([128, 2], F32, tag="rstdb")
                    nc.scalar.activation(
                        rstd[:, 0:1], mv[:, 1:2],
                        mybir.ActivationFunctionType.Ln, bias=eps_sb[:, 0:1],
                    )
                    nc.scalar.activation(
                        rstd[:, 1:2], rstd[:, 0:1],
                        mybir.ActivationFunctionType.Exp, scale=-0.5,
                    )
                    outt = ot_pool.tile([128, 512], BF16, tag="out")
                    nc.vector.tensor_scalar(
                        outt[:], r2[:],
                        mv[:, 0:1], rstd[:, 1:2],
                        op0=mybir.AluOpType.subtract, op1=mybir.AluOpType.mult,
                    )
                    nc.sync.dma_start(out_d[:, 512 * qt: 512 * qt + 512], outt[:])

                # software pipeline: FFN(qt-1) issues behind attention(qt)
                for qt in range(NQT):
                    attn_qt(qt)
                    if qt > 0:
                        ffn_qt(qt - 1)
                ffn_qt(NQT - 1)

    nc.compile()
    return nc


def _prep_host(inputs):
    x = np.asarray(inputs["x"], np.float32)
    conv_w = np.asarray(inputs["conv_w"], np.float32)
    conv_b = np.asarray(inputs["conv_b"], np.float32)
    in_w = np.asarray(inputs["in_proj_w"], np.float32)
    in_b = np.asarray(inputs["in_proj_b"], np.float32)
    out_w = np.asarray(inputs["out_proj_w"], np.float32)
    out_b = np.asarray(inputs["out_proj_b"], np.float32)
    w1 = np.asarray(inputs["w1"], np.float32)
    b1 = np.asarray(inputs["b1"], np.float32)
    w2 = np.asarray(inputs["w2"], np.float32)
    b2 = np.asarray(inputs["b2"], np.float32)
    g1 = np.asarray(inputs["ln1_g"], np.float32)
    bb1 = np.asarray(inputs["ln1_b"], np.float32)
    g2 = np.asarray(inputs["ln2_g"], np.float32)
    bb2 = np.asarray(inputs["ln2_b"], np.float32)

    for nm, v in (("conv_b", conv_b), ("in_proj_b", in_b), ("out_proj_b", out_b),
                  ("b1", b1), ("b2", b2)):
        if np.any(v != 0):
            raise NotImplementedError(f"nonzero {nm} unsupported")
    if np.any(g1 != 1) or np.any(bb1 != 0) or np.any(g2 != 1) or np.any(bb2 != 0):
        raise NotImplementedError("nontrivial layernorm affine unsupported")

    Wq, Wk, Wv = in_w[:D], in_w[D:2 * D], in_w[2 * D:]
    U = [(Wv @ conv_w[:, :, d]) for d in range(3)]  # v[t] = sum U_d @ x[t+d-1]

    def img(stack):  # [n, 128, w] slices -> [128, n*w] SBUF image
        a = np.asarray(stack, np.float32)
        return np.ascontiguousarray(a.transpose(1, 0, 2).reshape(128, -1))

    def slc16(W):  # W used as out = W @ x  -> lhsT slices of W.T
        WT = np.ascontiguousarray(W.T)
        return img([
            WT[128 * kc:128 * kc + 128, 128 * oc:128 * oc + 128]
            for kc in range(4) for oc in range(4)
        ])

    import ml_dtypes

    def bf(a):
        return np.asarray(a, dtype=ml_dtypes.bfloat16)

    def f8(a, s):
        return np.asarray(np.clip(np.asarray(a, np.float32) * s, -240, 240),
                          dtype=ml_dtypes.float8_e4m3)

    wk_a = f8(slc16(Wk), SWK)
    wq_a = f8(slc16(Wq), SWK)
    wu_a = f8(img([
        np.ascontiguousarray(U[tap].T)[128 * dc:128 * dc + 128, :]
        for tap in range(3) for dc in range(4)
    ]), SWU)
    woT_a = bf(img([out_w.T[128 * c:128 * c + 128, :] for c in range(4)]))
    w1_a = bf(img([
        np.ascontiguousarray(w1.T)[128 * dc:128 * dc + 128, 128 * fc:128 * fc + 128]
        for dc in range(4) for fc in range(8)
    ]))
    w2T_a = bf(img([w2.T[128 * fcb:128 * fcb + 128, :] for fcb in range(8)]))

    r = np.arange(128)
    m_lo = (r[:, None] >= r[None, :]).astype(np.float32)   # block 0: keep k>=q
    # additive masks, transposed for the lhsT.T @ identity trick
    negm = np.concatenate(
        [(-240.0 * (1.0 - m_lo)).T, (-240.0 * (1.0 - m_lo.T)).T], axis=1
    )

    ident = np.eye(128, dtype=np.float32)

    common = {
        "wk8": wk_a, "wq8": wq_a, "wu8": wu_a, "woT": woT_a,
        "w1": w1_a, "w2T": w2T_a, "negm": bf(negm), "ident": bf(ident),
    }

    in_maps = []
    for c in range(N_CORES):
        b, j = divmod(c, 4)
        s = 512 * j
        xe = np.zeros((XE, D), np.float32)
        lo, hi = max(0, s - 257), min(T, s + 769)
        xe[lo - (s - 257): hi - (s - 257)] = x[b, lo:hi]
        xt = xe.T.reshape(4, 128, XE).transpose(1, 0, 2)      # [128, 4, XE]
        xt = np.concatenate(
            [xt, np.zeros((128, 4, XE8 - XE), np.float32)], axis=2
        ).reshape(128, 4 * XE8)
        xt = np.ascontiguousarray(xt)

        xq = np.ascontiguousarray(
            x[b, s:s + 512].reshape(4, 128, 512).transpose(1, 0, 2).reshape(128, 2048)
        )

        # padcnt[qt, r]: in-band-kept pad keys
        key = (s - 256 + 128 * np.arange(4)[:, None, None]
               + np.arange(640)[None, None, :])          # [qt,1,640]
        pad = (key < 0) | (key >= T)
        cc, rr = np.arange(640)[None, None, :], r[None, :, None]
        kept = ((cc >= 128) & (cc < 512)) | ((cc < 128) & (cc >= rr)) \
            | ((cc >= 512) & (cc - 512 <= rr))
        pc = (pad & kept).sum(axis=2).astype(np.float32)  # [4, 128]
        padcnt = np.ascontiguousarray(pc.T)               # [128, 4]

        m = dict(common)
        m["xt8"] = f8(xt, SX)
        m["xq"] = bf(xq)
        m["padcnt"] = padcnt
        in_maps.append(m)
    return in_maps


def kernel(**inputs) -> np.ndarray:
    if "nc" not in _cached:
        _cached["nc"] = _build_program()
    nc = _cached["nc"]
    in_maps = _prep_host(inputs)
    res = run_bass_kernel_spmd(nc, in_maps, core_ids=list(range(N_CORES)))
    out = np.empty((B, T, D), np.float32)
    for c in range(N_CORES):
        b, j = divmod(c, 4)
        o = np.asarray(res.results[c]["out"], np.float32)
        o = o.reshape(128, 4, 512).transpose(1, 0, 2)
        out[b, 512 * j: 512 * j + 512] = o.reshape(512, 512)
    return out


# revision 11
# speedup vs baseline: 1.2356x; 1.0112x over previous
"""Trainium2 Bass kernel for nn_MicroExpert (sparse_attention).

Reference model (B=2, T=2048, D=512, H=8, HD=64):
  v_in = conv1d(x, k=3, pad=1); MHA(q=x, k=x, v=v_in) with banded mask
  |i-j| <= 256; h = LN(x + attn); out = LN(h + FFN(h)).

Sharding: data-parallel over (batch, 512-token chunk) -> 8 independent
cores, no collectives.  Each core recomputes the K/V halo (+-256 tokens,
zero-padded at sequence edges; pad keys are neutralized via a
denominator correction `padcnt`).

Key implementation points:
- K/Q/V projections run in fp8e4 DoubleRow (2 contraction tiles per
  instruction); per-tensor scales are folded into the psum->sbuf
  copies, so attention math downstream is plain bf16.
- The band mask is ADDED into the score psum as a -240 constant via two
  extra PE matmuls (lhsT=maskT, rhs=identity) closing each edge tile's
  accumulation group -- no vector-engine mask op, exp(score-30)~=0.
- The ctx matmul uses ex as the stationary operand so ctx lands
  TOKEN-major [q, vd] with the softmax denominator in column 64 of each
  head's 65-wide slot (ones-column trick); slots are packed 4-per-psum
  bank so no matmul output straddles a bank.  Normalization is one DVE
  multiply with a broadcast reciprocal-denominator operand.
- out_proj and FFN2 keep token-major outputs by streaming the weight as
  the moving operand, so residuals and both LayerNorms run token-major
  with only two PE transposes per 128-token tile.
- LN stats via DVE bn_stats/bn_aggr; rstd = exp(-0.5*ln(var+eps)) on
  ACT.  An explicit LoadActFuncSet pins the one table that holds Exp,
  Ln and Copy together so no table reloads occur mid-kernel.
- FFN(qt-1) is software-pipelined behind attention(qt); LN2 + store
  happen per query tile so the store overlaps compute.
- The conv is folded into the V projection on the host:
  v[t] = sum_d U_d @ x[t+d-1], U_d = Wv @ conv_w[:,:,d].
"""

import os
import sys

import numpy as np

sys.path.insert(0, "/opt/trn_rl_repo")

import concourse.bass as bass
import concourse.mybir as mybir
import concourse.tile as tile
from concourse import bacc
from concourse.bass_utils import run_bass_kernel_spmd

BF16 = mybir.dt.bfloat16
F32 = mybir.dt.float32
FP8 = mybir.dt.float8e4

B, T, D, H, HD = 2, 2048, 512, 8, 64
S = 512          # tokens per core
KV = 1024        # extended kv tokens per core (S + 2*256)
XE = 1026        # x_ext width (KV + 2 for conv halo)
XE8 = 1152       # fp8 image row stride: DoubleRow planes need stride % 128 == 0
NQT = 4          # 128-query tiles per core
NKT = 5          # relative 128-key tiles per query tile
F = 1024         # FFN hidden
EPS = 1e-5
N_CORES = 8

# fp8 quantization scales (powers of two; values stay well under 240)
SX = 32.0        # x
SWK = 1024.0     # Wk/Wq
SWU = 4096.0     # conv-folded V weight
SWO = 1024.0     # out_proj weight
SCTX = 1024.0    # normalized context (fp8 operand of out_proj)
ACT_TABLE_EXP_LN = 6   # act_info.json index of natural_log_exp_and_others

_cached = {}


def _build_program():
    nc = bacc.Bacc("TRN2", target_bir_lowering=False, debug=False)

    def din(name, shape, dt):
        return nc.dram_tensor(name, shape, dt, kind="ExternalInput").ap()

    # all inputs are pre-layouted [128, N] SBUF images (host does the packing)
    xt_d = din("xt8", [128, 4 * XE8], FP8)     # x extended, feature-major, *SX
    wk_d = din("wk8", [128, 2048], FP8)        # Wk.T (kc,oc) blocks, *SWK
    wq_d = din("wq8", [128, 2048], FP8)        # Wq.T (kc,oc) blocks, *SWK
    wu_d = din("wu8", [128, 6144], FP8)        # conv-folded V w (tap,dc), *SWU
    xq_d = din("xq", [128, 2048], BF16)        # x token-major (residual)
    woT_d = din("woT8", [128, 2048], FP8)      # Wo.T row-blocks [vd c][512], *SWO
    w1_d = din("w1", [128, 4096], BF16)        # w1.T (dc,fc) 128x128 blocks
    w2T_d = din("w2T", [128, 4096], BF16)      # w2.T row-blocks [fc][512]
    mask_d = din("mask01", [128, 256], BF16)   # [tril | triu] 0/1 keep masks
    padcnt_d = din("padcnt", [128, 4], F32)    # [q-in-tile, qt]
    ident_d = din("ident", [128, 128], BF16)

    out_d = nc.dram_tensor("out", [128, 2048], BF16, kind="ExternalOutput").ap()

    with tile.TileContext(nc) as tc:
        from contextlib import ExitStack

        with ExitStack() as ctx:
            const = ctx.enter_context(tc.tile_pool(name="const", bufs=1))

            # pin the act table that covers Exp+Ln+Copy for the whole kernel
            nc.scalar.add_instruction(mybir.InstLoadActFuncSet(
                name=nc.get_next_instruction_name(), ins=[], outs=[],
                act_func_set_id=ACT_TABLE_EXP_LN,
            ))

            # ---- loads: one DMA per tensor, split over both HWDGE engines,
            # in consumption order so compute starts as soon as possible
            def load_w(dram, cols, dt=BF16, eng=None):
                t = const.tile([128, cols], dt, name=f"w_{dram.tensor.name}")
                (eng or nc.sync).dma_start(t[:, :], dram[:, :])
                return t

            # ACT's preamble ends ~3us before SP's, so the tensors gating
            # the first matmuls all go on the ACT hwdge engine, K first.
            xt_sb = load_w(xt_d, 4 * XE8, FP8, nc.scalar)
            wk_sb = const.tile([128, 2048], FP8, name="w_wk8")
            for oc in range(4):
                nc.scalar.dma_start(
                    wk_sb[:, 512 * oc: 512 * oc + 512],
                    wk_d[:, 512 * oc: 512 * oc + 512],
                )
            wq_sb = load_w(wq_d, 2048, FP8, nc.scalar)
            wu_sb = load_w(wu_d, 6144, FP8, nc.scalar)
            ident_sb = load_w(ident_d, 128, BF16, nc.sync)
            mask_sb = load_w(mask_d, 256, BF16, nc.sync)
            padcnt_sb = load_w(padcnt_d, 4, F32, nc.sync)
            woT_sb = load_w(woT_d, 2048, FP8, nc.sync)
            xq_sb = load_w(xq_d, 2048, BF16, nc.sync)
            w1_sb = load_w(w1_d, 4096, BF16, nc.sync)
            w2T_sb = load_w(w2T_d, 4096, BF16, nc.sync)

            # persistent activations
            kt_sb = const.tile([128, 4 * KV], BF16)    # [oc-block][kv]
            q_sb = const.tile([128, 4 * S], BF16)      # [oc-block][tok]
            v_sb = const.tile([128, 8 * 520], BF16)    # [kv-tok][(v_h|1) x 8]
            hn_sb = const.tile([128, 4 * 512], BF16)   # [tok][qt-block][D]
            h1t_sb = const.tile([128, 4 * 512], BF16)  # [dc-block][tok]

            eps_sb = const.tile([128, 1], F32)
            nc.gpsimd.memset(v_sb[:], 1.0)
            nc.gpsimd.memset(eps_sb[:], float(EPS))
            h1t_v = h1t_sb[:].rearrange("p (c w) -> p c w", c=4)

            vx8 = xt_sb[:].rearrange("p (c w) -> p c w", c=4)      # [128,4,XE8]
            vwk = wk_sb[:].rearrange("p (o g w) -> p o g w", o=4, g=2)
            vwq = wq_sb[:].rearrange("p (o g w) -> p o g w", o=4, g=2)
            vwo8 = woT_sb[:].rearrange("p (c w) -> p c w", c=4)
            vwu = wu_sb[:].rearrange("p (t w) -> p t w", t=12)     # [128,12,512]
            DR = mybir.MatmulPerfMode.DoubleRow

            # ---- projections (fp8 DoubleRow): kT, qT, v ----------------
            with tc.tile_pool(name="pp", bufs=2, space="PSUM") as pp_pool:
                for oc in range(4):
                    for half in range(2):
                        pp = pp_pool.tile([128, 512], F32, tag="pp")
                        for g in range(2):
                            nc.tensor.matmul(
                                pp[:],
                                vwk[:, oc, g].rearrange("p (k w) -> p k w", k=2),
                                vx8[:, 2 * g: 2 * g + 2,
                                    1 + 512 * half: 513 + 512 * half],
                                start=(g == 0), stop=(g == 1), perf_mode=DR,
                            )
                        nc.scalar.mul(
                            kt_sb[:, KV * oc + 512 * half: KV * oc + 512 * half + 512],
                            pp[:], 1.0 / (SX * SWK),
                        )
                for oc in range(4):
                    pp = pp_pool.tile([128, 512], F32, tag="pp")
                    for g in range(2):
                        nc.tensor.matmul(
                            pp[:],
                            vwq[:, oc, g].rearrange("p (k w) -> p k w", k=2),
                            vx8[:, 2 * g: 2 * g + 2, 257:769],
                            start=(g == 0), stop=(g == 1), perf_mode=DR,
                        )
                    nc.vector.tensor_scalar_mul(
                        q_sb[:, 512 * oc: 512 * oc + 512], pp[:], 1.0 / (SX * SWK)
                    )
                for tt in range(8):
                    pp = pp_pool.tile([128, 512], F32, tag="pp")
                    n = 0
                    for tap in range(3):
                        for i in range(2):
                            nc.tensor.matmul(
                                pp[:],
                                vx8[:, 2 * i: 2 * i + 2,
                                    128 * tt + tap: 128 * tt + tap + 128],
                                vwu[:, 4 * tap + 2 * i: 4 * tap + 2 * i + 2, :],
                                start=(n == 0), stop=(n == 5), perf_mode=DR,
                            )
                            n += 1
                    vv = v_sb[:, 520 * tt: 520 * tt + 520].rearrange(
                        "p (h w) -> p h w", h=8
                    )
                    nc.scalar.mul(
                        vv[:, :, 0:64],
                        pp[:].rearrange("p (h w) -> p h w", h=8),
                        1.0 / (SX * SWU),
                    )

            # ---- attention + LN1 (+ software-pipelined FFN) ------------
            with ExitStack() as actx:
                sc_pool = actx.enter_context(tc.tile_pool(name="scps", bufs=2, space="PSUM"))
                cx_pool = actx.enter_context(tc.tile_pool(name="cxps", bufs=1, space="PSUM"))
                tp_pool = actx.enter_context(tc.tile_pool(name="tpps", bufs=1, space="PSUM"))
                ps_pool = actx.enter_context(tc.tile_pool(name="ps1", bufs=2, space="PSUM"))
                ex_pool = actx.enter_context(tc.tile_pool(name="exsb", bufs=3))
                cn_pool = actx.enter_context(tc.tile_pool(name="cnsb", bufs=2))
                r1_pool = actx.enter_context(tc.tile_pool(name="r1sb", bufs=2))
                sm_pool = actx.enter_context(tc.tile_pool(name="smsb", bufs=4))
                ms_pool = actx.enter_context(tc.tile_pool(name="m1sb", bufs=2))
                r2_pool = actx.enter_context(tc.tile_pool(name="r2sb", bufs=2))
                ot_pool = actx.enter_context(tc.tile_pool(name="otsb", bufs=2))

                def attn_qt(qt):
                    ctxn = cn_pool.tile([128, 512], BF16, tag="ctxn")
                    for g in range(2):
                        cxps = cx_pool.tile([128, 260], F32, tag="cx")
                        for h4 in range(4):
                            h = 4 * g + h4
                            oc, hp = h // 2, 64 * (h % 2)
                            scps = sc_pool.tile([128, 640], F32, tag="sc")
                            for kt in range(NKT):
                                kcol = KV * oc + 128 * (qt + kt)
                                nc.tensor.matmul(
                                    scps[:, 128 * kt: 128 * kt + 128],
                                    kt_sb[hp:hp + 64, kcol:kcol + 128],
                                    q_sb[hp:hp + 64, 512 * oc + 128 * qt: 512 * oc + 128 * qt + 128],
                                    start=True, stop=True,
                                )
                            ex = ex_pool.tile([128, 640], BF16, tag="ex")
                            nc.scalar.activation(
                                ex[:], scps[:], mybir.ActivationFunctionType.Exp,
                                scale=float(1.0 / np.sqrt(HD)),
                            )
                            # inner ctx tiles start right after the exp
                            for kt in (1, 2, 3):
                                nc.tensor.matmul(
                                    cxps[:, 65 * h4: 65 * h4 + 65],
                                    ex[:, 128 * kt: 128 * kt + 128],
                                    v_sb[:, 520 * (qt + kt) + 65 * h: 520 * (qt + kt) + 65 * h + 65],
                                    start=(kt == 1), stop=False,
                                    skip_group_check=True,
                                )
                            # 0/1 band mask on the edge tiles (DVE/Pool split)
                            ex_edge = ex[:].rearrange("p (a b) -> p a b", a=5)[:, ::4, :]
                            meng = nc.vector if h % 2 == 0 else nc.gpsimd
                            meng.tensor_mul(
                                ex_edge, ex_edge,
                                mask_sb[:].rearrange("p (n w) -> p n w", n=2),
                            )
                            for kt in (0, 4):
                                nc.tensor.matmul(
                                    cxps[:, 65 * h4: 65 * h4 + 65],
                                    ex[:, 128 * kt: 128 * kt + 128],
                                    v_sb[:, 520 * (qt + kt) + 65 * h: 520 * (qt + kt) + 65 * h + 65],
                                    start=False, stop=(kt == 4),
                                    skip_group_check=True,
                                )
                        # denominators (col 64 per slot) minus pad count
                        cxv = cxps[:].rearrange("p (h w) -> p h w", h=4)
                        den = sm_pool.tile([128, 4], F32, tag="den")
                        nc.vector.tensor_scalar_sub(
                            den[:].rearrange("p (h w) -> p h w", w=1),
                            cxv[:, :, 64:65], padcnt_sb[:, qt:qt + 1],
                        )
                        rden = sm_pool.tile([128, 4], F32, tag="rden")
                        nc.vector.reciprocal(rden[:], den[:])
                        # normalize this half -> token-major [q, vd] bf16
                        rdv = rden[:].rearrange(
                            "p (h w) -> p h w", w=1
                        ).to_broadcast((128, 4, 64))
                        nc.vector.tensor_mul(
                            ctxn[:, 256 * g: 256 * g + 256].rearrange(
                                "p (h w) -> p h w", h=4),
                            cxv[:, :, 0:64], rdv,
                        )
                    # transpose -> [vd, q]; quantize to fp8 for DR out_proj
                    tpps = tp_pool.tile([128, 512], BF16, tag="tp")
                    for c in range(4):
                        nc.tensor.transpose(
                            tpps[:, 128 * c: 128 * c + 128],
                            ctxn[:, 128 * c: 128 * c + 128],
                            ident_sb[:],
                        )
                    ctxT = cn_pool.tile([128, 512], FP8, tag="ctxT")
                    nc.scalar.mul(ctxT[:], tpps[:], float(SCTX))
                    ctxTv = ctxT[:].rearrange("p (c w) -> p c w", c=4)
                    # out_proj (token-major out, fp8 DoubleRow) + residual
                    atps = ps_pool.tile([128, 512], F32, tag="ps")
                    for g in range(2):
                        nc.tensor.matmul(
                            atps[:],
                            ctxTv[:, 2 * g: 2 * g + 2, :],
                            vwo8[:, 2 * g: 2 * g + 2, :],
                            start=(g == 0), stop=(g == 1), perf_mode=DR,
                        )
                    r1 = r1_pool.tile([128, 512], BF16, tag="r1")
                    nc.vector.scalar_tensor_tensor(
                        r1[:], atps[:], 1.0 / (SCTX * SWO),
                        xq_sb[:, 512 * qt: 512 * qt + 512],
                        op0=mybir.AluOpType.mult, op1=mybir.AluOpType.add,
                    )
                    # LN1: bn_stats/bn_aggr; rstd = exp(-0.5*ln(var+eps))
                    bn6 = sm_pool.tile([128, 6], F32, tag="bn6")
                    nc.vector.bn_stats(bn6[:], r1[:])
                    mv = sm_pool.tile([128, 2], F32, tag="mv")
                    nc.vector.bn_aggr(mv[:], bn6[:])
                    rstd = sm_pool.tile([128, 2], F32, tag="rstd")
                    nc.scalar.activation(
                        rstd[:, 0:1], mv[:, 1:2],
                        mybir.ActivationFunctionType.Ln, bias=eps_sb[:, 0:1],
                    )
                    nc.scalar.activation(
                        rstd[:, 1:2], rstd[:, 0:1],
                        mybir.ActivationFunctionType.Exp, scale=-0.5,
                    )
                    nc.vector.tensor_scalar(
                        hn_sb[:, 512 * qt: 512 * qt + 512], r1[:],
                        mv[:, 0:1], rstd[:, 1:2],
                        op0=mybir.AluOpType.subtract, op1=mybir.AluOpType.mult,
                    )
                    # hn -> feature-major for FFN1 via DMA-XBAR transpose;
                    # latency hidden by the qt-level software pipeline
                    for c in range(4):
                        nc.sync.dma_start_transpose(
                            h1t_v[:, c, 128 * qt: 128 * qt + 128],
                            hn_sb[:, 512 * qt + 128 * c: 512 * qt + 128 * c + 128],
                        )

                def ffn_qt(qt):
                    m1sb = ms_pool.tile([128, 1024], BF16, tag="m1")
                    for g in range(2):
                        m1ps = ps_pool.tile([128, 512], F32, tag="ps")
                        for i in range(4):
                            fc = 4 * g + i
                            for dc in range(4):
                                nc.tensor.matmul(
                                    m1ps[:, 128 * i: 128 * i + 128],
                                    w1_sb[:, 128 * (8 * dc + fc): 128 * (8 * dc + fc) + 128],
                                    h1t_v[:, dc, 128 * qt: 128 * qt + 128],
                                    start=(dc == 0), stop=(dc == 3),
                                )
                        nc.vector.tensor_scalar_max(
                            m1sb[:, 512 * g: 512 * g + 512], m1ps[:], 0.0
                        )
                    f2ps = ps_pool.tile([128, 512], F32, tag="ps")
                    for fcb in range(8):
                        nc.tensor.matmul(
                            f2ps[:],
                            m1sb[:, 128 * fcb: 128 * fcb + 128],
                            w2T_sb[:, 512 * fcb: 512 * fcb + 512],
                            start=(fcb == 0), stop=(fcb == 7),
                        )
                    r2 = r2_pool.tile([128, 512], BF16, tag="r2")
                    nc.vector.tensor_add(
                        r2[:], f2ps[:], hn_sb[:, 512 * qt: 512 * qt + 512]
                    )
                    bn6 = sm_pool.tile([128, 6], F32, tag="bn6b")
                    nc.vector.bn_stats(bn6[:], r2[:])
                    mv = sm_pool.tile([128, 2], F32, tag="mvb")
                    nc.vector.bn_aggr(mv[:], bn6[:])
                    rstd = sm_pool.tile([128, 2], F32, tag="rstdb")
                    nc.scalar.activation(
                        rstd[:, 0:1], mv[:, 1:2],
                        mybir.ActivationFunctionType.Ln, bias=eps_sb[:, 0:1],
                    )
                    nc.scalar.activation(
                        rstd[:, 1:2], rstd[:, 0:1],
                        mybir.ActivationFunctionType.Exp, scale=-0.5,
                    )
                    outt = ot_pool.tile([128, 512], BF16, tag="out")
                    nc.vector.tensor_scalar(
                        outt[:], r2[:],
                        mv[:, 0:1], rstd[:, 1:2],
                        op0=mybir.AluOpType.subtract, op1=mybir.AluOpType.mult,
                    )
                    nc.sync.dma_start(out_d[:, 512 * qt: 512 * qt + 512], outt[:])

                # software pipeline: FFN(qt-1) issues behind attention(qt)
                for qt in range(NQT):
                    attn_qt(qt)
                    if qt > 0:
                        ffn_qt(qt - 1)
                ffn_qt(NQT - 1)

    nc.compile()
    return nc


def _prep_host(inputs):
    x = np.asarray(inputs["x"], np.float32)
    conv_w = np.asarray(inputs["conv_w"], np.float32)
    conv_b = np.asarray(inputs["conv_b"], np.float32)
    in_w = np.asarray(inputs["in_proj_w"], np.float32)
    in_b = np.asarray(inputs["in_proj_b"], np.float32)
    out_w = np.asarray(inputs["out_proj_w"], np.float32)
    out_b = np.asarray(inputs["out_proj_b"], np.float32)
    w1 = np.asarray(inputs["w1"], np.float32)
    b1 = np.asarray(inputs["b1"], np.float32)
    w2 = np.asarray(inputs["w2"], np.float32)
    b2 = np.asarray(inputs["b2"], np.float32)
    g1 = np.asarray(inputs["ln1_g"], np.float32)
    bb1 = np.asarray(inputs["ln1_b"], np.float32)
    g2 = np.asarray(inputs["ln2_g"], np.float32)
    bb2 = np.asarray(inputs["ln2_b"], np.float32)

    for nm, v in (("conv_b", conv_b), ("in_proj_b", in_b), ("out_proj_b", out_b),
                  ("b1", b1), ("b2", b2)):
        if np.any(v != 0):
            raise NotImplementedError(f"nonzero {nm} unsupported")
    if np.any(g1 != 1) or np.any(bb1 != 0) or np.any(g2 != 1) or np.any(bb2 != 0):
        raise NotImplementedError("nontrivial layernorm affine unsupported")

    Wq, Wk, Wv = in_w[:D], in_w[D:2 * D], in_w[2 * D:]
    U = [(Wv @ conv_w[:, :, d]) for d in range(3)]  # v[t] = sum U_d @ x[t+d-1]

    def img(stack):  # [n, 128, w] slices -> [128, n*w] SBUF image
        a = np.asarray(stack, np.float32)
        return np.ascontiguousarray(a.transpose(1, 0, 2).reshape(128, -1))

    def slc16(W):  # W used as out = W @ x  -> lhsT slices of W.T
        WT = np.ascontiguousarray(W.T)
        return img([
            WT[128 * kc:128 * kc + 128, 128 * oc:128 * oc + 128]
            for kc in range(4) for oc in range(4)
        ])

    import ml_dtypes

    def bf(a):
        return np.asarray(a, dtype=ml_dtypes.bfloat16)

    def f8(a, s):
        return np.asarray(np.clip(np.asarray(a, np.float32) * s, -240, 240),
                          dtype=ml_dtypes.float8_e4m3)

    def slc_ocg(W):  # oc-major (oc, g, k2) 128x128 blocks of W.T
        WT = np.ascontiguousarray(W.T)
        return img([
            WT[128 * (2 * g + k2):128 * (2 * g + k2) + 128, 128 * oc:128 * oc + 128]
            for oc in range(4) for g in range(2) for k2 in range(2)
        ])

    wk_a = f8(slc_ocg(Wk), SWK)
    wq_a = f8(slc_ocg(Wq), SWK)
    wu_a = f8(img([
        np.ascontiguousarray(U[tap].T)[128 * dc:128 * dc + 128, :]
        for tap in range(3) for dc in range(4)
    ]), SWU)
    woT_a = f8(img([out_w.T[128 * c:128 * c + 128, :] for c in range(4)]), SWO)
    w1_a = bf(img([
        np.ascontiguousarray(w1.T)[128 * dc:128 * dc + 128, 128 * fc:128 * fc + 128]
        for dc in range(4) for fc in range(8)
    ]))
    w2T_a = bf(img([w2.T[128 * fcb:128 * fcb + 128, :] for fcb in range(8)]))

    r = np.arange(128)
    m_lo = (r[:, None] >= r[None, :]).astype(np.float32)   # block 0: keep k>=q
    mask01 = np.concatenate([m_lo, m_lo.T], axis=1)

    ident = np.eye(128, dtype=np.float32)

    common = {
        "wk8": wk_a, "wq8": wq_a, "wu8": wu_a, "woT8": woT_a,
        "w1": w1_a, "w2T": w2T_a, "mask01": bf(mask01), "ident": bf(ident),
    }

    in_maps = []
    for c in range(N_CORES):
        b, j = divmod(c, 4)
        s = 512 * j
        xe = np.zeros((XE, D), np.float32)
        lo, hi = max(0, s - 257), min(T, s + 769)
        xe[lo - (s - 257): hi - (s - 257)] = x[b, lo:hi]
        xt = xe.T.reshape(4, 128, XE).transpose(1, 0, 2)      # [128, 4, XE]
        xt = np.concatenate(
            [xt, np.zeros((128, 4, XE8 - XE), np.float32)], axis=2
        ).reshape(128, 4 * XE8)
        xt = np.ascontiguousarray(xt)

        xq = np.ascontiguousarray(
            x[b, s:s + 512].reshape(4, 128, 512).transpose(1, 0, 2).reshape(128, 2048)
        )

        # padcnt[qt, r]: in-band-kept pad keys
        key = (s - 256 + 128 * np.arange(4)[:, None, None]
               + np.arange(640)[None, None, :])          # [qt,1,640]
        pad = (key < 0) | (key >= T)
        cc, rr = np.arange(640)[None, None, :], r[None, :, None]
        kept = ((cc >= 128) & (cc < 512)) | ((cc < 128) & (cc >= rr)) \
            | ((cc >= 512) & (cc - 512 <= rr))
        pc = (pad & kept).sum(axis=2).astype(np.float32)  # [4, 128]
        padcnt = np.ascontiguousarray(pc.T)               # [128, 4]

        m = dict(common)
        m["xt8"] = f8(xt, SX)
        m["xq"] = bf(xq)
        m["padcnt"] = padcnt
        in_maps.append(m)
    return in_maps


def kernel(**inputs) -> np.ndarray:
    if "nc" not in _cached:
        _cached["nc"] = _build_program()
    nc = _cached["nc"]
    in_maps = _prep_host(inputs)
    res = run_bass_kernel_spmd(nc, in_maps, core_ids=list(range(N_CORES)))
    out = np.empty((B, T, D), np.float32)
    for c in range(N_CORES):
        b, j = divmod(c, 4)
        o = np.asarray(res.results[c]["out"], np.float32)
        o = o.reshape(128, 4, 512).transpose(1, 0, 2)
        out[b, 512 * j: 512 * j + 512] = o.reshape(512, 512)
    return out


# revision 12
# speedup vs baseline: 1.2609x; 1.0204x over previous
"""Trainium2 Bass kernel for nn_MicroExpert (sparse_attention).

Reference model (B=2, T=2048, D=512, H=8, HD=64):
  v_in = conv1d(x, k=3, pad=1); MHA(q=x, k=x, v=v_in) with banded mask
  |i-j| <= 256; h = LN(x + attn); out = LN(h + FFN(h)).

Sharding: data-parallel over (batch, 512-token chunk) -> 8 independent
cores, no collectives.  Each core recomputes the K/V halo (+-256 tokens,
zero-padded at sequence edges; pad keys are neutralized via a
denominator correction `padcnt`).

Key implementation points:
- K/Q/V projections run in fp8e4 DoubleRow (2 contraction tiles per
  instruction); per-tensor scales are folded into the psum->sbuf
  copies, so attention math downstream is plain bf16.
- The band mask is ADDED into the score psum as a -240 constant via two
  extra PE matmuls (lhsT=maskT, rhs=identity) closing each edge tile's
  accumulation group -- no vector-engine mask op, exp(score-30)~=0.
- The ctx matmul uses ex as the stationary operand so ctx lands
  TOKEN-major [q, vd] with the softmax denominator in column 64 of each
  head's 65-wide slot (ones-column trick); slots are packed 4-per-psum
  bank so no matmul output straddles a bank.  Normalization is one DVE
  multiply with a broadcast reciprocal-denominator operand.
- out_proj and FFN2 keep token-major outputs by streaming the weight as
  the moving operand, so residuals and both LayerNorms run token-major
  with only two PE transposes per 128-token tile.
- LN stats via DVE bn_stats/bn_aggr; rstd = exp(-0.5*ln(var+eps)) on
  ACT.  An explicit LoadActFuncSet pins the one table that holds Exp,
  Ln and Copy together so no table reloads occur mid-kernel.
- FFN(qt-1) is software-pipelined behind attention(qt); LN2 + store
  happen per query tile so the store overlaps compute.
- The conv is folded into the V projection on the host:
  v[t] = sum_d U_d @ x[t+d-1], U_d = Wv @ conv_w[:,:,d].
"""

import os
import sys

import numpy as np

sys.path.insert(0, "/opt/trn_rl_repo")

import concourse.bass as bass
import concourse.mybir as mybir
import concourse.tile as tile
from concourse import bacc
from concourse.bass_utils import run_bass_kernel_spmd

BF16 = mybir.dt.bfloat16
F32 = mybir.dt.float32
FP8 = mybir.dt.float8e4

B, T, D, H, HD = 2, 2048, 512, 8, 64
S = 512          # tokens per core
KV = 1024        # extended kv tokens per core (S + 2*256)
XE = 1026        # x_ext width (KV + 2 for conv halo)
XE8 = 1152       # fp8 image row stride: DoubleRow planes need stride % 128 == 0
NQT = 4          # 128-query tiles per core
NKT = 5          # relative 128-key tiles per query tile
F = 1024         # FFN hidden
EPS = 1e-5
N_CORES = 8

# fp8 quantization scales (powers of two; values stay well under 240)
SX = 32.0        # x
SWK = 1024.0     # Wk/Wq
SWU = 4096.0     # conv-folded V weight
SWO = 1024.0     # out_proj weight
SCTX = 1024.0    # normalized context (fp8 operand of out_proj)
ACT_TABLE_EXP_LN = 6   # act_info.json index of natural_log_exp_and_others

_cached = {}


def _build_program():
    nc = bacc.Bacc("TRN2", target_bir_lowering=False, debug=False)

    def din(name, shape, dt):
        return nc.dram_tensor(name, shape, dt, kind="ExternalInput").ap()

    # inputs are packed into two contiguous "walls" (one DMA each) plus a
    # tiny f32 tensor; host packs in the same order.
    # wall A (fp8): xt8 | wk8 | wq8 | wu8 | woT8
    WA_XT, WA_WK, WA_WQ, WA_WU, WA_WO = 0, 4 * XE8, 4 * XE8 + 2048, 4 * XE8 + 4096, 4 * XE8 + 10240
    WA_COLS = 4 * XE8 + 12288
    # wall B (bf16): xq | w1 | w2T | mask01 | ident
    WB_XQ, WB_W1, WB_W2, WB_MK, WB_ID = 0, 2048, 6144, 10240, 10496
    WB_COLS = 10624
    wa_d = din("wallA", [128, WA_COLS], FP8)
    wb_d = din("wallB", [128, WB_COLS], BF16)
    padcnt_d = din("padcnt", [128, 4], F32)    # [q-in-tile, qt]

    out_d = nc.dram_tensor("out", [128, 2048], BF16, kind="ExternalOutput").ap()

    with tile.TileContext(nc) as tc:
        from contextlib import ExitStack

        with ExitStack() as ctx:
            const = ctx.enter_context(tc.tile_pool(name="const", bufs=1))

            # pin the act table that covers Exp+Ln+Copy for the whole kernel
            nc.scalar.add_instruction(mybir.InstLoadActFuncSet(
                name=nc.get_next_instruction_name(), ins=[], outs=[],
                act_func_set_id=ACT_TABLE_EXP_LN,
            ))

            # ---- loads: one DMA per tensor, split over both HWDGE engines,
            # in consumption order so compute starts as soon as possible
            def load_w(dram, cols, dt=BF16, eng=None):
                t = const.tile([128, cols], dt, name=f"w_{dram.tensor.name}")
                (eng or nc.sync).dma_start(t[:, :], dram[:, :])
                return t

            # ACT's preamble ends ~3us before SP's: wall A (everything the
            # projections need) goes on ACT as one big DMA; wall B on SP.
            wa_sb = load_w(wa_d, WA_COLS, FP8, nc.scalar)
            wb_sb = load_w(wb_d, WB_COLS, BF16, nc.sync)
            padcnt_sb = load_w(padcnt_d, 4, F32, nc.sync)
            xt_sb = wa_sb[:, WA_XT: WA_XT + 4 * XE8]
            wk_sb = wa_sb[:, WA_WK: WA_WK + 2048]
            wq_sb = wa_sb[:, WA_WQ: WA_WQ + 2048]
            wu_sb = wa_sb[:, WA_WU: WA_WU + 6144]
            woT_sb = wa_sb[:, WA_WO: WA_WO + 2048]
            xq_sb = wb_sb[:, WB_XQ: WB_XQ + 2048]
            w1_sb = wb_sb[:, WB_W1: WB_W1 + 4096]
            w2T_sb = wb_sb[:, WB_W2: WB_W2 + 4096]
            mask_sb = wb_sb[:, WB_MK: WB_MK + 256]
            ident_sb = wb_sb[:, WB_ID: WB_ID + 128]

            # persistent activations
            kt_sb = const.tile([128, 4 * KV], BF16)    # [oc-block][kv]
            q_sb = const.tile([128, 4 * S], BF16)      # [oc-block][tok]
            v_sb = const.tile([128, 8 * 520], BF16)    # [kv-tok][(v_h|1) x 8]
            hn_sb = const.tile([128, 4 * 512], BF16)   # [tok][qt-block][D]
            h1t_sb = const.tile([128, 4 * 512], BF16)  # [dc-block][tok]

            eps_sb = const.tile([128, 1], F32)
            nc.gpsimd.memset(v_sb[:], 1.0)
            nc.gpsimd.memset(eps_sb[:], float(EPS))
            h1t_v = h1t_sb[:].rearrange("p (c w) -> p c w", c=4)

            vx8 = xt_sb.rearrange("p (c w) -> p c w", c=4)      # [128,4,XE8]
            vwk = wk_sb.rearrange("p (o g w) -> p o g w", o=4, g=2)
            vwq = wq_sb.rearrange("p (o g w) -> p o g w", o=4, g=2)
            vwo8 = woT_sb.rearrange("p (c w) -> p c w", c=4)
            vwu = wu_sb.rearrange("p (t w) -> p t w", t=12)     # [128,12,512]
            DR = mybir.MatmulPerfMode.DoubleRow

            # ---- projections (fp8 DoubleRow): kT, qT, v ----------------
            with tc.tile_pool(name="pp", bufs=2, space="PSUM") as pp_pool:
                for oc in range(4):
                    for half in range(2):
                        pp = pp_pool.tile([128, 512], F32, tag="pp")
                        for g in range(2):
                            nc.tensor.matmul(
                                pp[:],
                                vwk[:, oc, g].rearrange("p (k w) -> p k w", k=2),
                                vx8[:, 2 * g: 2 * g + 2,
                                    1 + 512 * half: 513 + 512 * half],
                                start=(g == 0), stop=(g == 1), perf_mode=DR,
                            )
                        nc.scalar.mul(
                            kt_sb[:, KV * oc + 512 * half: KV * oc + 512 * half + 512],
                            pp[:], 1.0 / (SX * SWK),
                        )
                for oc in range(4):
                    pp = pp_pool.tile([128, 512], F32, tag="pp")
                    for g in range(2):
                        nc.tensor.matmul(
                            pp[:],
                            vwq[:, oc, g].rearrange("p (k w) -> p k w", k=2),
                            vx8[:, 2 * g: 2 * g + 2, 257:769],
                            start=(g == 0), stop=(g == 1), perf_mode=DR,
                        )
                    nc.vector.tensor_scalar_mul(
                        q_sb[:, 512 * oc: 512 * oc + 512], pp[:], 1.0 / (SX * SWK)
                    )
                for tt in range(8):
                    pp = pp_pool.tile([128, 512], F32, tag="pp")
                    n = 0
                    for tap in range(3):
                        for i in range(2):
                            nc.tensor.matmul(
                                pp[:],
                                vx8[:, 2 * i: 2 * i + 2,
                                    128 * tt + tap: 128 * tt + tap + 128],
                                vwu[:, 4 * tap + 2 * i: 4 * tap + 2 * i + 2, :],
                                start=(n == 0), stop=(n == 5), perf_mode=DR,
                            )
                            n += 1
                    vv = v_sb[:, 520 * tt: 520 * tt + 520].rearrange(
                        "p (h w) -> p h w", h=8
                    )
                    nc.scalar.mul(
                        vv[:, :, 0:64],
                        pp[:].rearrange("p (h w) -> p h w", h=8),
                        1.0 / (SX * SWU),
                    )

            # ---- attention + LN1 (+ software-pipelined FFN) ------------
            with ExitStack() as actx:
                sc_pool = actx.enter_context(tc.tile_pool(name="scps", bufs=2, space="PSUM"))
                cx_pool = actx.enter_context(tc.tile_pool(name="cxps", bufs=1, space="PSUM"))
                tp_pool = actx.enter_context(tc.tile_pool(name="tpps", bufs=1, space="PSUM"))
                ps_pool = actx.enter_context(tc.tile_pool(name="ps1", bufs=2, space="PSUM"))
                ex_pool = actx.enter_context(tc.tile_pool(name="exsb", bufs=3))
                cn_pool = actx.enter_context(tc.tile_pool(name="cnsb", bufs=2))
                r1_pool = actx.enter_context(tc.tile_pool(name="r1sb", bufs=2))
                sm_pool = actx.enter_context(tc.tile_pool(name="smsb", bufs=4))
                ms_pool = actx.enter_context(tc.tile_pool(name="m1sb", bufs=2))
                r2_pool = actx.enter_context(tc.tile_pool(name="r2sb", bufs=2))
                ot_pool = actx.enter_context(tc.tile_pool(name="otsb", bufs=2))

                def attn_qt(qt):
                    ctxn = cn_pool.tile([128, 512], BF16, tag="ctxn")
                    for g in range(2):
                        cxps = cx_pool.tile([128, 260], F32, tag="cx")
                        for h4 in range(4):
                            h = 4 * g + h4
                            oc, hp = h // 2, 64 * (h % 2)
                            scps = sc_pool.tile([128, 640], F32, tag="sc")
                            for kt in range(NKT):
                                kcol = KV * oc + 128 * (qt + kt)
                                nc.tensor.matmul(
                                    scps[:, 128 * kt: 128 * kt + 128],
                                    kt_sb[hp:hp + 64, kcol:kcol + 128],
                                    q_sb[hp:hp + 64, 512 * oc + 128 * qt: 512 * oc + 128 * qt + 128],
                                    start=True, stop=True,
                                )
                            ex = ex_pool.tile([128, 640], BF16, tag="ex")
                            nc.scalar.activation(
                                ex[:], scps[:], mybir.ActivationFunctionType.Exp,
                                scale=float(1.0 / np.sqrt(HD)),
                            )
                            # 0/1 band mask on edge tiles, then one unbroken
                            # 5-matmul ctx accumulation group on the PE
                            ex_edge = ex[:].rearrange("p (a b) -> p a b", a=5)[:, ::4, :]
                            nc.vector.tensor_mul(
                                ex_edge, ex_edge,
                                mask_sb.rearrange("p (n w) -> p n w", n=2),
                            )
                            for kt in range(NKT):
                                nc.tensor.matmul(
                                    cxps[:, 65 * h4: 65 * h4 + 65],
                                    ex[:, 128 * kt: 128 * kt + 128],
                                    v_sb[:, 520 * (qt + kt) + 65 * h: 520 * (qt + kt) + 65 * h + 65],
                                    start=(kt == 0), stop=(kt == 4),
                                    skip_group_check=True,
                                )
                        # denominators (col 64 per slot) minus pad count
                        cxv = cxps[:].rearrange("p (h w) -> p h w", h=4)
                        den = sm_pool.tile([128, 4], F32, tag="den")
                        nc.vector.tensor_scalar_sub(
                            den[:].rearrange("p (h w) -> p h w", w=1),
                            cxv[:, :, 64:65], padcnt_sb[:, qt:qt + 1],
                        )
                        rden = sm_pool.tile([128, 4], F32, tag="rden")
                        nc.vector.reciprocal(rden[:], den[:])
                        # normalize this half -> token-major [q, vd] bf16
                        rdv = rden[:].rearrange(
                            "p (h w) -> p h w", w=1
                        ).to_broadcast((128, 4, 64))
                        nc.vector.tensor_mul(
                            ctxn[:, 256 * g: 256 * g + 256].rearrange(
                                "p (h w) -> p h w", h=4),
                            cxv[:, :, 0:64], rdv,
                        )
                    # transpose -> [vd, q]; quantize to fp8 for DR out_proj
                    tpps = tp_pool.tile([128, 512], BF16, tag="tp")
                    for c in range(4):
                        nc.tensor.transpose(
                            tpps[:, 128 * c: 128 * c + 128],
                            ctxn[:, 128 * c: 128 * c + 128],
                            ident_sb,
                        )
                    ctxT = cn_pool.tile([128, 512], FP8, tag="ctxT")
                    nc.scalar.mul(ctxT[:], tpps[:], float(SCTX))
                    ctxTv = ctxT[:].rearrange("p (c w) -> p c w", c=4)
                    # out_proj (token-major out, fp8 DoubleRow) + residual
                    atps = ps_pool.tile([128, 512], F32, tag="ps")
                    for g in range(2):
                        nc.tensor.matmul(
                            atps[:],
                            ctxTv[:, 2 * g: 2 * g + 2, :],
                            vwo8[:, 2 * g: 2 * g + 2, :],
                            start=(g == 0), stop=(g == 1), perf_mode=DR,
                        )
                    r1 = r1_pool.tile([128, 512], BF16, tag="r1")
                    nc.vector.scalar_tensor_tensor(
                        r1[:], atps[:], 1.0 / (SCTX * SWO),
                        xq_sb[:, 512 * qt: 512 * qt + 512],
                        op0=mybir.AluOpType.mult, op1=mybir.AluOpType.add,
                    )
                    # LN1: bn_stats/bn_aggr; rstd = exp(-0.5*ln(var+eps))
                    bn6 = sm_pool.tile([128, 6], F32, tag="bn6")
                    nc.vector.bn_stats(bn6[:], r1[:])
                    mv = sm_pool.tile([128, 2], F32, tag="mv")
                    nc.vector.bn_aggr(mv[:], bn6[:])
                    rstd = sm_pool.tile([128, 2], F32, tag="rstd")
                    nc.scalar.activation(
                        rstd[:, 0:1], mv[:, 1:2],
                        mybir.ActivationFunctionType.Ln, bias=eps_sb[:, 0:1],
                    )
                    nc.scalar.activation(
                        rstd[:, 1:2], rstd[:, 0:1],
                        mybir.ActivationFunctionType.Exp, scale=-0.5,
                    )
                    nc.vector.tensor_scalar(
                        hn_sb[:, 512 * qt: 512 * qt + 512], r1[:],
                        mv[:, 0:1], rstd[:, 1:2],
                        op0=mybir.AluOpType.subtract, op1=mybir.AluOpType.mult,
                    )
                    # hn -> feature-major for FFN1.  For qt<3 the XBAR DMA
                    # transpose is free (latency hidden by the software
                    # pipeline); the drain tile uses the low-latency PE path.
                    if qt < NQT - 1:
                        for c in range(4):
                            nc.sync.dma_start_transpose(
                                h1t_v[:, c, 128 * qt: 128 * qt + 128],
                                hn_sb[:, 512 * qt + 128 * c: 512 * qt + 128 * c + 128],
                            )
                    else:
                        tpps2 = tp_pool.tile([128, 512], BF16, tag="tp")
                        for c in range(4):
                            nc.tensor.transpose(
                                tpps2[:, 128 * c: 128 * c + 128],
                                hn_sb[:, 512 * qt + 128 * c: 512 * qt + 128 * c + 128],
                                ident_sb,
                            )
                        nc.vector.tensor_copy(
                            h1t_v[:, :, 128 * qt: 128 * qt + 128], tpps2[:]
                        )

                def ffn_qt(qt):
                    m1sb = ms_pool.tile([128, 1024], BF16, tag="m1")
                    for g in range(2):
                        m1ps = ps_pool.tile([128, 512], F32, tag="ps")
                        for i in range(4):
                            fc = 4 * g + i
                            for dc in range(4):
                                nc.tensor.matmul(
                                    m1ps[:, 128 * i: 128 * i + 128],
                                    w1_sb[:, 128 * (8 * dc + fc): 128 * (8 * dc + fc) + 128],
                                    h1t_v[:, dc, 128 * qt: 128 * qt + 128],
                                    start=(dc == 0), stop=(dc == 3),
                                )
                        nc.vector.tensor_scalar_max(
                            m1sb[:, 512 * g: 512 * g + 512], m1ps[:], 0.0
                        )
                    f2ps = ps_pool.tile([128, 512], F32, tag="ps")
                    for fcb in range(8):
                        nc.tensor.matmul(
                            f2ps[:],
                            m1sb[:, 128 * fcb: 128 * fcb + 128],
                            w2T_sb[:, 512 * fcb: 512 * fcb + 512],
                            start=(fcb == 0), stop=(fcb == 7),
                        )
                    r2 = r2_pool.tile([128, 512], BF16, tag="r2")
                    nc.vector.tensor_add(
                        r2[:], f2ps[:], hn_sb[:, 512 * qt: 512 * qt + 512]
                    )
                    bn6 = sm_pool.tile([128, 6], F32, tag="bn6b")
                    nc.vector.bn_stats(bn6[:], r2[:])
                    mv = sm_pool.tile([128, 2], F32, tag="mvb")
                    nc.vector.bn_aggr(mv[:], bn6[:])
                    rstd = sm_pool.tile([128, 2], F32, tag="rstdb")
                    nc.scalar.activation(
                        rstd[:, 0:1], mv[:, 1:2],
                        mybir.ActivationFunctionType.Ln, bias=eps_sb[:, 0:1],
                    )
                    nc.scalar.activation(
                        rstd[:, 1:2], rstd[:, 0:1],
                        mybir.ActivationFunctionType.Exp, scale=-0.5,
                    )
                    outt = ot_pool.tile([128, 512], BF16, tag="out")
                    nc.vector.tensor_scalar(
                        outt[:], r2[:],
                        mv[:, 0:1], rstd[:, 1:2],
                        op0=mybir.AluOpType.subtract, op1=mybir.AluOpType.mult,
                    )
                    nc.sync.dma_start(out_d[:, 512 * qt: 512 * qt + 512], outt[:])

                # software pipeline: FFN(qt-1) issues behind attention(qt)
                for qt in range(NQT):
                    attn_qt(qt)
                    if qt > 0:
                        ffn_qt(qt - 1)
                ffn_qt(NQT - 1)

    nc.compile()
    return nc


def _prep_host(inputs):
    x = np.asarray(inputs["x"], np.float32)
    conv_w = np.asarray(inputs["conv_w"], np.float32)
    conv_b = np.asarray(inputs["conv_b"], np.float32)
    in_w = np.asarray(inputs["in_proj_w"], np.float32)
    in_b = np.asarray(inputs["in_proj_b"], np.float32)
    out_w = np.asarray(inputs["out_proj_w"], np.float32)
    out_b = np.asarray(inputs["out_proj_b"], np.float32)
    w1 = np.asarray(inputs["w1"], np.float32)
    b1 = np.asarray(inputs["b1"], np.float32)
    w2 = np.asarray(inputs["w2"], np.float32)
    b2 = np.asarray(inputs["b2"], np.float32)
    g1 = np.asarray(inputs["ln1_g"], np.float32)
    bb1 = np.asarray(inputs["ln1_b"], np.float32)
    g2 = np.asarray(inputs["ln2_g"], np.float32)
    bb2 = np.asarray(inputs["ln2_b"], np.float32)

    for nm, v in (("conv_b", conv_b), ("in_proj_b", in_b), ("out_proj_b", out_b),
                  ("b1", b1), ("b2", b2)):
        if np.any(v != 0):
            raise NotImplementedError(f"nonzero {nm} unsupported")
    if np.any(g1 != 1) or np.any(bb1 != 0) or np.any(g2 != 1) or np.any(bb2 != 0):
        raise NotImplementedError("nontrivial layernorm affine unsupported")

    Wq, Wk, Wv = in_w[:D], in_w[D:2 * D], in_w[2 * D:]
    U = [(Wv @ conv_w[:, :, d]) for d in range(3)]  # v[t] = sum U_d @ x[t+d-1]

    def img(stack):  # [n, 128, w] slices -> [128, n*w] SBUF image
        a = np.asarray(stack, np.float32)
        return np.ascontiguousarray(a.transpose(1, 0, 2).reshape(128, -1))

    def slc16(W):  # W used as out = W @ x  -> lhsT slices of W.T
        WT = np.ascontiguousarray(W.T)
        return img([
            WT[128 * kc:128 * kc + 128, 128 * oc:128 * oc + 128]
            for kc in range(4) for oc in range(4)
        ])

    import ml_dtypes

    def bf(a):
        return np.asarray(a, dtype=ml_dtypes.bfloat16)

    def f8(a, s):
        return np.asarray(np.clip(np.asarray(a, np.float32) * s, -240, 240),
                          dtype=ml_dtypes.float8_e4m3)

    def slc_ocg(W):  # oc-major (oc, g, k2) 128x128 blocks of W.T
        WT = np.ascontiguousarray(W.T)
        return img([
            WT[128 * (2 * g + k2):128 * (2 * g + k2) + 128, 128 * oc:128 * oc + 128]
            for oc in range(4) for g in range(2) for k2 in range(2)
        ])

    wk_a = f8(slc_ocg(Wk), SWK)
    wq_a = f8(slc_ocg(Wq), SWK)
    wu_a = f8(img([
        np.ascontiguousarray(U[tap].T)[128 * dc:128 * dc + 128, :]
        for tap in range(3) for dc in range(4)
    ]), SWU)
    woT_a = f8(img([out_w.T[128 * c:128 * c + 128, :] for c in range(4)]), SWO)
    w1_a = bf(img([
        np.ascontiguousarray(w1.T)[128 * dc:128 * dc + 128, 128 * fc:128 * fc + 128]
        for dc in range(4) for fc in range(8)
    ]))
    w2T_a = bf(img([w2.T[128 * fcb:128 * fcb + 128, :] for fcb in range(8)]))

    r = np.arange(128)
    m_lo = (r[:, None] >= r[None, :]).astype(np.float32)   # block 0: keep k>=q
    mask01 = np.concatenate([m_lo, m_lo.T], axis=1)

    ident = np.eye(128, dtype=np.float32)

    wallA_w = np.concatenate([wk_a, wq_a, wu_a, woT_a], axis=1)
    wallB = bf(np.concatenate(
        [np.zeros((128, 2048), np.float32), w1_a.astype(np.float32),
         w2T_a.astype(np.float32), mask01, ident], axis=1))
    common = {"wallA_w": wallA_w, "wallB": wallB}

    in_maps = []
    for c in range(N_CORES):
        b, j = divmod(c, 4)
        s = 512 * j
        xe = np.zeros((XE, D), np.float32)
        lo, hi = max(0, s - 257), min(T, s + 769)
        xe[lo - (s - 257): hi - (s - 257)] = x[b, lo:hi]
        xt = xe.T.reshape(4, 128, XE).transpose(1, 0, 2)      # [128, 4, XE]
        xt = np.concatenate(
            [xt, np.zeros((128, 4, XE8 - XE), np.float32)], axis=2
        ).reshape(128, 4 * XE8)
        xt = np.ascontiguousarray(xt)

        xq = np.ascontiguousarray(
            x[b, s:s + 512].reshape(4, 128, 512).transpose(1, 0, 2).reshape(128, 2048)
        )

        # padcnt[qt, r]: in-band-kept pad keys
        key = (s - 256 + 128 * np.arange(4)[:, None, None]
               + np.arange(640)[None, None, :])          # [qt,1,640]
        pad = (key < 0) | (key >= T)
        cc, rr = np.arange(640)[None, None, :], r[None, :, None]
        kept = ((cc >= 128) & (cc < 512)) | ((cc < 128) & (cc >= rr)) \
            | ((cc >= 512) & (cc - 512 <= rr))
        pc = (pad & kept).sum(axis=2).astype(np.float32)  # [4, 128]
        padcnt = np.ascontiguousarray(pc.T)               # [128, 4]

        wallA = np.concatenate([f8(xt, SX), common["wallA_w"]], axis=1)
        wallB = common["wallB"].copy()
        wallB[:, 0:2048] = bf(xq)
        m = {"wallA": wallA, "wallB": wallB, "padcnt": padcnt}
        in_maps.append(m)
    return in_maps


def kernel(**inputs) -> np.ndarray:
    if "nc" not in _cached:
        _cached["nc"] = _build_program()
    nc = _cached["nc"]
    in_maps = _prep_host(inputs)
    res = run_bass_kernel_spmd(nc, in_maps, core_ids=list(range(N_CORES)))
    out = np.empty((B, T, D), np.float32)
    for c in range(N_CORES):
        b, j = divmod(c, 4)
        o = np.asarray(res.results[c]["out"], np.float32)
        o = o.reshape(128, 4, 512).transpose(1, 0, 2)
        out[b, 512 * j: 512 * j + 512] = o.reshape(512, 512)
    return out
